# revision 6
# baseline (speedup 1.0000x reference)
"""ArteryMixer Trainium2 kernel: 8-core data-parallel over tokens.

Per-token math (B=2,S=2048,A=8,R=8,DIM=1024,H=8,HD=128,SC=16):
  qkv = concat(x+emb, res) @ Wqkv.T ; q,k rmsnorm ; k_res roped (folded into W);
  scores=elu(q@k.T/sqrt(HD)) ; mixed = scores@v/16 ; out = x + scale*(mixed@Wproj.T)

Device layout strategy (per core, 512 tokens):
  - All activations kept FEATURE-on-partitions (x.T etc., host pre-transposed).
  - QKV (q,k_art,k_res·Rope) via W-stationary GEMM -> qT/kT slabs (d-part, col=(t,slot)).
  - V via X-stationary GEMM -> v slabs in row layout (rows=(t,slot) on partitions).
  - artery-embed bias folded in as rank-8 extra matmul (one-hot trick).
  - rmsnorm: square (DVE) + gpsimd partition_all_reduce (f32 internal) + ACT ln/exp rsqrt,
    scale folded: rs_q = rsqrt(ssq/128+eps), rs_k = rsqrt(ssq+128*eps) (= rs*HD^-0.5).
  - attention per 16-token group: scoresT = kT_slice.T @ qT_slice (cross products),
    elu via Relu(ACT)+min(exp-1,0), block-diag mask*(1/16) kills cross-token terms.
  - mixedT = v.T @ routeT accumulated art+res -> feature-part layout feeds proj GEMM.
  - proj W-stationary -> projT ; y.T = projT*mixer_scale + x.T ; output stays transposed,
    host un-transposes.
"""

import numpy as np
import ml_dtypes

bf16 = ml_dtypes.bfloat16

HEADS = 8
HD = 128
DIM = 1024
MD = 1024
A = 8
RKV = 8
SC = 16
EPS = 1.1920929e-07
ROPE_BASE = 10000.0
N_CORES = 8
B, S = 2, 2048
TOK_PER_CORE = (B * S) // N_CORES  # 512
BLK_TOK = 64                        # tokens per pipeline block
NB = TOK_PER_CORE // BLK_TOK        # 8 blocks
CPB = BLK_TOK * 8                   # 512 cols per block (token-major, slot-minor)


def _rope_matrix():
    inv_freq = 1.0 / (ROPE_BASE ** (np.arange(0, HD, 2, dtype=np.float64) / HD))
    c, s = np.cos(inv_freq), np.sin(inv_freq)
    Rm = np.zeros((HD, HD), dtype=np.float64)
    i = np.arange(HD // 2)
    # reference _rope: out1 = x1*c + x2*s ; out2 = -x1*s + x2*c
    Rm[i, i] = c
    Rm[i, i + 64] = s
    Rm[i + 64, i] = -s
    Rm[i + 64, i + 64] = c
    return Rm


def build_program(tok_per_core=TOK_PER_CORE):
    import concourse.bass as bass  # noqa
    import concourse.mybir as mybir
    import concourse.tile as tile
    from concourse import bacc
    from concourse import bass_isa

    dt = mybir.dt
    Alu = mybir.AluOpType
    Act = mybir.ActivationFunctionType

    nb = tok_per_core // BLK_TOK
    COLS = tok_per_core * 8

    nc = bacc.Bacc(None, target_bir_lowering=False)

    xt_art = nc.dram_tensor("xt_art", [DIM, COLS], dt.bfloat16, kind="ExternalInput")
    xt_res = nc.dram_tensor("xt_res", [DIM, COLS], dt.bfloat16, kind="ExternalInput")
    wqkv_t = nc.dram_tensor("wqkv_t", [DIM, 3 * MD], dt.bfloat16, kind="ExternalInput")
    wv_t = nc.dram_tensor("wv_t", [DIM, MD], dt.bfloat16, kind="ExternalInput")
    wproj_t = nc.dram_tensor("wproj_t", [MD, DIM], dt.bfloat16, kind="ExternalInput")
    biasqk_d = nc.dram_tensor("biasqk", [8, 2 * MD], dt.bfloat16, kind="ExternalInput")
    biasv_d = nc.dram_tensor("biasv", [8, MD], dt.bfloat16, kind="ExternalInput")
    onehot_d = nc.dram_tensor("onehot", [8, CPB], dt.bfloat16, kind="ExternalInput")
    mask_d = nc.dram_tensor("mask", [128, 128], dt.bfloat16, kind="ExternalInput")
    mscale_d = nc.dram_tensor("mscale", [128, 8], dt.float32, kind="ExternalInput")
    out_t = nc.dram_tensor("out_t", [DIM, COLS], dt.bfloat16, kind="ExternalOutput")

    with tile.TileContext(nc) as tc:
        with (
            tc.tile_pool(name="w", bufs=1) as wpool,
            tc.tile_pool(name="x", bufs=1) as xpool,
            tc.tile_pool(name="slab", bufs=1) as spool,
            tc.tile_pool(name="nrm", bufs=2) as npool,
            tc.tile_pool(name="att", bufs=2) as fpool,
            tc.tile_pool(name="y", bufs=1) as ypool,
            tc.tile_pool(name="mm", bufs=2, space="PSUM") as mmpool,
            tc.tile_pool(name="sc", bufs=2, space="PSUM") as scpool,
            tc.tile_pool(name="mx", bufs=1, space="PSUM") as mxpool,
        ):
            # ---- resident weights/constants ----
            wqkv_sb = wpool.tile([128, 8, 3 * MD], dt.bfloat16)
            nc.sync.dma_start(
                wqkv_sb, wqkv_t[:].rearrange("(dc p) f -> p dc f", p=128)
            )
            wv_sb = wpool.tile([128, 8, MD], dt.bfloat16)
            nc.sync.dma_start(wv_sb, wv_t[:].rearrange("(dc p) f -> p dc f", p=128))
            wproj_sb = wpool.tile([128, 8, DIM], dt.bfloat16)
            nc.sync.dma_start(
                wproj_sb, wproj_t[:].rearrange("(mc p) f -> p mc f", p=128)
            )
            biasqk_sb = wpool.tile([8, 2 * MD], dt.bfloat16)
            nc.sync.dma_start(biasqk_sb, biasqk_d[:])
            biasv_sb = wpool.tile([8, MD], dt.bfloat16)
            nc.sync.dma_start(biasv_sb, biasv_d[:])
            onehot_sb = wpool.tile([8, CPB], dt.bfloat16)
            nc.sync.dma_start(onehot_sb, onehot_d[:])
            mask_sb = wpool.tile([128, 128], dt.bfloat16)
            nc.sync.dma_start(mask_sb, mask_d[:])
            mscale_sb = wpool.tile([128, 8], dt.float32)
            nc.sync.dma_start(mscale_sb, mscale_d[:])
            eps_q = wpool.tile([128, 1], dt.float32)
            nc.vector.memset(eps_q, EPS)
            eps_k = wpool.tile([128, 1], dt.float32)
            nc.vector.memset(eps_k, HD * EPS)

            xa_dram = xt_art[:].rearrange("(dc p) c -> p dc c", p=128)
            xr_dram = xt_res[:].rearrange("(dc p) c -> p dc c", p=128)
            yo_dram = out_t[:].rearrange("(dc p) c -> p dc c", p=128)

            for blk in range(nb):
                c0 = blk * CPB
                xa = xpool.tile([128, 8, CPB], dt.bfloat16, tag="xa")
                xr = xpool.tile([128, 8, CPB], dt.bfloat16, tag="xr")
                nc.sync.dma_start(xa, xa_dram[:, :, c0 : c0 + CPB])
                nc.sync.dma_start(xr, xr_dram[:, :, c0 : c0 + CPB])

                qT = spool.tile([128, 8, CPB], dt.bfloat16, tag="qT")
                kTa = spool.tile([128, 8, CPB], dt.bfloat16, tag="kTa")
                kTr = spool.tile([128, 8, CPB], dt.bfloat16, tag="kTr")
                slabs = [qT, kTa, kTr]

                # ---- QKV GEMM (W-stationary): fc 0-7 q, 8-15 k_art, 16-23 k_res ----
                for fc in range(24):
                    ps = mmpool.tile([128, CPB], dt.float32, tag="mmps")
                    src = xr if fc >= 16 else xa
                    has_bias = fc < 16
                    for dc in range(8):
                        nc.tensor.matmul(
                            ps,
                            wqkv_sb[:, dc, fc * 128 : (fc + 1) * 128],
                            src[:, dc, :],
                            start=(dc == 0),
                            stop=(dc == 7 and not has_bias),
                        )
                    if has_bias:
                        nc.tensor.matmul(
                            ps,
                            biasqk_sb[:, fc * 128 : (fc + 1) * 128],
                            onehot_sb[:, :CPB],
                            start=False,
                            stop=True,
                        )
                    nc.scalar.copy(out=slabs[fc // 8][:, fc % 8, :], in_=ps)

                # ---- V GEMM (X-stationary, row-layout out) ----
                va = spool.tile([128, 4, 8, HD], dt.bfloat16, tag="va")
                vr = spool.tile([128, 4, 8, HD], dt.bfloat16, tag="vr")
                for src, dstv, has_bias in ((xa, va, True), (xr, vr, False)):
                    for rc in range(4):
                        for vh in range(2):
                            ps = mmpool.tile([128, 512], dt.float32, tag="mmps")
                            for dc in range(8):
                                nc.tensor.matmul(
                                    ps,
                                    src[:, dc, rc * 128 : (rc + 1) * 128],
                                    wv_sb[:, dc, vh * 512 : (vh + 1) * 512],
                                    start=(dc == 0),
                                    stop=(dc == 7 and not has_bias),
                                )
                            if has_bias:
                                nc.tensor.matmul(
                                    ps,
                                    onehot_sb[:, :128],
                                    biasv_sb[:, vh * 512 : (vh + 1) * 512],
                                    start=False,
                                    stop=True,
                                )
                            nc.scalar.copy(
                                out=dstv[:, rc, vh * 4 : (vh + 1) * 4, :], in_=ps
                            )

                # ---- RMS norm (scale folded: q gets /sqrt(HD) via k-side fold) ----
                for slab, epsv, scv in (
                    (qT, eps_q, 1.0 / HD),
                    (kTa, eps_k, 1.0),
                    (kTr, eps_k, 1.0),
                ):
                    for hh in range(4):  # quarter-slab passes
                        sl = slab[:, hh * 2 : (hh + 1) * 2, :]
                        sq = npool.tile([128, 2, CPB], dt.float32, tag="nsq")
                        nc.vector.tensor_mul(sq, sl, sl)
                        ssq = npool.tile([128, 2, CPB], dt.float32, tag="nssq")
                        nc.gpsimd.partition_all_reduce(
                            ssq, sq, channels=128, reduce_op=bass_isa.ReduceOp.add
                        )
                        # rsqrt(scv*ssq+eps) = exp(-0.5*ln(scv*ssq+eps)); Ln+Exp share
                        # one ACT table set (avoids Sqrt-set thrash)
                        rs = npool.tile([128, 2, CPB], dt.float32, tag="nsq")
                        nc.scalar.activation(rs, ssq, Act.Ln, bias=epsv, scale=scv)
                        rs2 = npool.tile([128, 2, CPB], dt.float32, tag="nssq")
                        nc.scalar.activation(rs2, rs, Act.Exp, scale=-0.5)
                        nc.vector.tensor_mul(sl, sl, rs2)

                # ---- attention: 4 groups of 16 tokens ----
                mixedT = spool.tile([128, 8, CPB], dt.bfloat16, tag="mixedT")
                for g in range(4):
                    gsl = slice(g * 128, (g + 1) * 128)
                    routes = []
                    for half, kT in enumerate((kTa, kTr)):
                        ps = scpool.tile([128, 8, 128], dt.float32, tag="scps")
                        for h in range(8):
                            nc.tensor.matmul(
                                ps[:, h, :],
                                kT[:, h, gsl],
                                qT[:, h, gsl],
                                start=True,
                                stop=True,
                            )
                        esc = fpool.tile([128, 8, 128], dt.float32, tag="esc")
                        rsc = fpool.tile([128, 8, 128], dt.float32, tag="rsc")
                        nc.scalar.activation(esc, ps, Act.Exp)
                        nc.scalar.activation(rsc, ps, Act.Relu)
                        # elu = relu(s) + min(exp(s)-1, 0) = relu(s) + (min(exp(s),1) - 1)
                        nc.vector.tensor_scalar(
                            esc, esc, 1.0, -1.0, Alu.min, Alu.add
                        )
                        nc.vector.tensor_add(esc, rsc, esc)
                        route = fpool.tile([128, 8, 128], dt.bfloat16, tag=f"rt{half}")
                        nc.vector.tensor_mul(
                            route, esc, mask_sb[:, None, :].to_broadcast((128, 8, 128))
                        )
                        routes.append(route)
                    mx = mxpool.tile([128, 8, 128], dt.float32, tag="mxps")
                    for h in range(8):
                        nc.tensor.matmul(
                            mx[:, h, :],
                            va[:, g, h, :],
                            routes[0][:, h, :],
                            start=True,
                            stop=False,
                        )
                        nc.tensor.matmul(
                            mx[:, h, :],
                            vr[:, g, h, :],
                            routes[1][:, h, :],
                            start=False,
                            stop=True,
                        )
                    nc.scalar.copy(out=mixedT[:, :, gsl], in_=mx)

                # ---- proj GEMM (W-stationary) + final y.T = projT*scale + x.T ----
                yb = ypool.tile([128, 8, CPB], dt.bfloat16, tag="yb")
                for dc in range(8):
                    ps = mmpool.tile([128, CPB], dt.float32, tag="mmps")
                    for h in range(8):
                        nc.tensor.matmul(
                            ps,
                            wproj_sb[:, h, dc * 128 : (dc + 1) * 128],
                            mixedT[:, h, :],
                            start=(h == 0),
                            stop=(h == 7),
                        )
                    nc.vector.scalar_tensor_tensor(
                        out=yb[:, dc, :],
                        in0=ps,
                        scalar=mscale_sb[:, dc : dc + 1],
                        in1=xa[:, dc, :],
                        op0=Alu.mult,
                        op1=Alu.add,
                    )
                nc.sync.dma_start(yo_dram[:, :, c0 : c0 + CPB], yb)

    nc.compile()
    return nc


def host_prep(x, artery_embed, residual_kv, Wqkv, Wproj, mixer_scale,
              tok_per_core=TOK_PER_CORE, n_cores=N_CORES):
    T = x.shape[0] * x.shape[1]
    x_flat = np.asarray(x, dtype=np.float32).reshape(T, A, DIM)
    res_flat = np.asarray(residual_kv, dtype=np.float32).reshape(T, RKV, DIM)

    Rm = _rope_matrix()
    Wq = np.asarray(Wqkv[0:MD], dtype=np.float64)
    Wk = np.asarray(Wqkv[MD : 2 * MD], dtype=np.float64)
    Wv = np.asarray(Wqkv[2 * MD : 3 * MD], dtype=np.float64)
    Wk_res = np.einsum("de,hec->hdc", Rm, Wk.reshape(HEADS, HD, DIM)).reshape(MD, DIM)

    wqkv_t = np.ascontiguousarray(
        np.concatenate([Wq, Wk, Wk_res], axis=0).T
    ).astype(bf16)
    wv_t = np.ascontiguousarray(Wv.T).astype(bf16)
    wproj_t = np.ascontiguousarray(np.asarray(Wproj, dtype=np.float64).T).astype(bf16)

    emb = np.asarray(artery_embed, dtype=np.float64)
    bias_q = emb @ Wq.T
    bias_k = emb @ Wk.T
    bias_v = emb @ Wv.T
    biasqk = np.ascontiguousarray(
        np.concatenate([bias_q, bias_k], axis=1)
    ).astype(bf16)
    biasv = np.ascontiguousarray(bias_v).astype(bf16)

    onehot = np.zeros((8, CPB), dtype=bf16)
    onehot[np.arange(CPB) % 8, np.arange(CPB)] = 1

    mask = np.zeros((128, 128), dtype=np.float32)
    for t in range(16):
        mask[t * 8 : (t + 1) * 8, t * 8 : (t + 1) * 8] = 1.0 / SC
    mask = mask.astype(bf16)

    mscale = np.ascontiguousarray(
        np.asarray(mixer_scale, dtype=np.float32).reshape(8, 128).T
    )

    shared = dict(
        wqkv_t=wqkv_t, wv_t=wv_t, wproj_t=wproj_t, biasqk=biasqk, biasv=biasv,
        onehot=onehot, mask=mask, mscale=mscale,
    )
    in_maps = []
    for i in range(n_cores):
        sl = slice(i * tok_per_core, (i + 1) * tok_per_core)
        xa = np.ascontiguousarray(
            x_flat[sl].reshape(tok_per_core * A, DIM).T
        ).astype(bf16)
        xr = np.ascontiguousarray(
            res_flat[sl].reshape(tok_per_core * RKV, DIM).T
        ).astype(bf16)
        m = dict(shared)
        m["xt_art"] = xa
        m["xt_res"] = xr
        in_maps.append(m)
    return in_maps


def assemble_output(outs, tok_per_core=TOK_PER_CORE):
    """outs: list of (DIM, tok_per_core*8) bf16 arrays -> (B,S,A,DIM) f32."""
    parts = []
    for o in outs:
        y = np.asarray(o, dtype=np.float32)  # (1024, T*8)
        parts.append(y.reshape(DIM, tok_per_core, A).transpose(1, 2, 0))
    full = np.concatenate(parts, axis=0)  # (n_tok, A, DIM)
    if full.shape[0] == B * S:
        full = full.reshape(B, S, A, DIM)
    return np.ascontiguousarray(full)


_NC_CACHE = {}


def kernel(x, artery_embed, residual_kv, Wqkv, Wproj, mixer_scale):
    from concourse.bass_utils import run_bass_kernel_spmd

    key = TOK_PER_CORE
    if key not in _NC_CACHE:
        _NC_CACHE[key] = build_program(TOK_PER_CORE)
    nc = _NC_CACHE[key]

    in_maps = host_prep(x, artery_embed, residual_kv, Wqkv, Wproj, mixer_scale)
    res = run_bass_kernel_spmd(nc, in_maps, core_ids=list(range(N_CORES)))
    outs = [r["out_t"] for r in res.results]
    return assemble_output(outs)
